# revision 1
# baseline (speedup 1.0000x reference)
"""Trainium2 Bass kernel for nn_EquivariantVelocityHead.

Full-input contract: kernel(**inputs) takes the unsharded inputs (as in
setup_inputs()) and returns the full [B*N, 3] output. Internally shards
data-parallel over the graph dimension B across 8 NeuronCores (all pairwise
interactions are intra-graph), with the tiny phi-MLP weights replicated.

Math (per graph, N=256 nodes, H=128):
  A = h @ W1[:H];  Bm = h @ W1[H:2H];  wd = W1[2H];  (phi layer 1 split)
  pre[p,q,:] = A[q] + Bm[p] + dist[p,q]*wd + b1
  coeff[p,q] = silu(pre) @ W2 + b2
  v[p] = sum_q coeff[p,q] * (pos[q] - pos[p])
       = coeff @ pos - rowsum(coeff) * pos[p]

Device layout: pre is materialized transposed [h=128 partitions, (p,q) free]
in PSUM by two accumulating fp32r matmuls per 2-node block:
  1) A-term: lhsT=Wa, rhs = hT with a stride-0 broadcast access pattern.
  2) B+dist term (K=3): stationary rows [wd; B[2t]; B[2t+1]] (built on
     device, staged to partitions 0-2), moving rows [dist; ind_even;
     ind_odd] where dist rows are DMA-staged onto partition 0 and the
     indicator rows select which half of the block each B row lands in.
Silu+b1 is fused on ScalarE reading PSUM. The W2 contraction uses a
sliding-window stationary (zeros | W2-column | zeros) so node p's block
accumulates into PSUM partition p%128, yielding coeff[p-part, q-free]
tiles directly; the final contraction runs on VectorE accumulating reduces.
"""
import sys

sys.path.insert(0, "/opt/trn_rl_repo")

import numpy as np

B, N, H = 8, 256, 128
NCORES = 8

_cache = {}


def _build(reps=1, variant="full"):
    import concourse.bacc as bacc
    import concourse.mybir as mybir
    import concourse.tile as tile

    F32 = mybir.dt.float32
    F32R = mybir.dt.float32r
    Alu = mybir.AluOpType
    Act = mybir.ActivationFunctionType

    nc = bacc.Bacc()

    hT_d = nc.declare_dram_parameter("hT", [H, N], F32R, isOutput=False)
    pos_d = nc.declare_dram_parameter("pos", [N, 3], F32, isOutput=False)
    rep_d = nc.declare_dram_parameter("rep3", [3, 128, N], F32, isOutput=False)
    wa_d = nc.declare_dram_parameter("wa", [H, H], F32R, isOutput=False)
    wb_d = nc.declare_dram_parameter("wb", [H, H], F32R, isOutput=False)
    wdrep_d = nc.declare_dram_parameter("wdrep", [1, 128 * H], F32R,
                                        isOutput=False)
    ind_d = nc.declare_dram_parameter("ind", [2, 4096], F32R, isOutput=False)
    zw_d = nc.declare_dram_parameter("zw", [H, 2 * H], F32R, isOutput=False)
    b1c_d = nc.declare_dram_parameter("b1c", [H, 1], F32, isOutput=False)
    b2c_d = nc.declare_dram_parameter("b2c", [128, 1], F32, isOutput=False)
    v_d = nc.declare_dram_parameter("v", [N, 3], F32, isOutput=True)

    with tile.TileContext(nc) as tc:
        with (
            tc.tile_pool(name="const", bufs=1) as cpool,
            tc.tile_pool(name="work", bufs=2) as wpool,
            tc.tile_pool(name="stage", bufs=8) as spool,
            tc.tile_pool(name="silu", bufs=4) as lpool,
            tc.tile_pool(name="fin", bufs=2) as fpool,
            tc.tile_pool(name="pre", bufs=3, space="PSUM") as pre_pool,
            tc.tile_pool(name="cps", bufs=1, space="PSUM") as cps_pool,
            tc.tile_pool(name="bps", bufs=1, space="PSUM") as bps_pool,
        ):
            # ---- constants / inputs ----
            hT = cpool.tile([H, N], F32R, tag="hT")
            nc.sync.dma_start(hT[:], hT_d[:])
            wa = cpool.tile([H, H], F32R, tag="wa")
            nc.sync.dma_start(wa[:], wa_d[:])
            wb = cpool.tile([H, H], F32R, tag="wb")
            nc.sync.dma_start(wb[:], wb_d[:])
            zw = cpool.tile([H, 2 * H], F32R, tag="zw")
            nc.sync.dma_start(zw[:], zw_d[:])
            b1c = cpool.tile([H, 1], F32, tag="b1c")
            nc.sync.dma_start(b1c[:], b1c_d[:])
            b2c = cpool.tile([128, 1], F32, tag="b2c")
            nc.sync.dma_start(b2c[:], b2c_d[:])
            rep = []
            for a in range(3):
                r = cpool.tile([128, N], F32, tag=f"rep{a}")
                nc.sync.dma_start(r[:], rep_d[a])
                rep.append(r)
            pcol = []
            for t in range(2):
                p = cpool.tile([128, 3], F32, tag=f"pcol{t}")
                nc.sync.dma_start(p[:], pos_d[128 * t:128 * (t + 1), :])
                pcol.append(p)
            bwd = cpool.tile([3, 128 * H], F32R, tag="bwd")
            nc.sync.dma_start(bwd[0:1, :], wdrep_d[:])

            for rp in range(reps):
                # ---- Bm = h @ Wb, scattered into bwd partitions 1-2 ----
                bsb = []
                for t in range(2):
                    bp = bps_pool.tile([128, H], F32, tag="bps")
                    nc.tensor.matmul(bp[:], hT[:, 128 * t:128 * (t + 1)],
                                     wb[:], start=True, stop=True,
                                     skip_group_check=True)
                    bs = wpool.tile([128, H], F32R, tag="bsb",
                                    name=f"bsb{t}_{rp}")
                    nc.vector.tensor_copy(bs[:], bp[:])
                    bsb.append(bs)
                for t in range(2):
                    dst = bwd[1 + t:2 + t, :]
                    nc.sync.dma_start(
                        dst.rearrange("o (a c) -> o a c", c=H), bsb[t][:])

                # ---- dist tiles [p-part, q-free], exact diff formulation ----
                dist = []
                for t in range(2):
                    dx = wpool.tile([128, N], F32, tag="dx", name=f"dx{t}_{rp}")
                    dy = wpool.tile([128, N], F32, tag="dy", name=f"dy{t}_{rp}")
                    dz = wpool.tile([128, N], F32, tag="dz", name=f"dz{t}_{rp}")
                    nc.vector.tensor_scalar(dx[:], rep[0][:], pcol[t][:, 0:1],
                                            None, Alu.subtract)
                    nc.vector.tensor_scalar(dy[:], rep[1][:], pcol[t][:, 1:2],
                                            None, Alu.subtract)
                    nc.vector.tensor_scalar(dz[:], rep[2][:], pcol[t][:, 2:3],
                                            None, Alu.subtract)
                    sx = wpool.tile([128, N], F32, tag="sx", name=f"sx{t}_{rp}")
                    sy = wpool.tile([128, N], F32, tag="sy", name=f"sy{t}_{rp}")
                    nc.vector.tensor_tensor(sx[:], dx[:], dx[:], Alu.mult)
                    nc.vector.tensor_tensor(sy[:], dy[:], dy[:], Alu.mult)
                    nc.vector.tensor_tensor(sx[:], sx[:], sy[:], Alu.add)
                    nc.vector.tensor_tensor(sy[:], dz[:], dz[:], Alu.mult)
                    nc.vector.tensor_tensor(sx[:], sx[:], sy[:], Alu.add)
                    dt_ = wpool.tile([128, N], F32R, tag="dist",
                                     name=f"dist{t}_{rp}")
                    nc.scalar.activation(dt_[:], sx[:], Act.Sqrt)
                    dist.append(dt_)

                # ---- stage: [dist rows; indicators] on partitions 0-2 ----
                stages = []
                for c in range(32):
                    st = spool.tile([3, 8 * N], F32R, tag="stage",
                                    name=f"stage{c}_{rp}")
                    row = st[0:1, :].rearrange("o (r two q) -> o r two q",
                                               two=2, q=N)
                    nc.sync.dma_start(row[:, :, 0, :],
                                      dist[0][4 * c:4 * c + 4, :])
                    nc.sync.dma_start(row[:, :, 1, :],
                                      dist[1][4 * c:4 * c + 4, :])
                    nc.sync.dma_start(st[1:3, :], ind_d[:, 0:2048])
                    stages.append(st)

                # ---- coeff accumulator: cols 0:256 -> nodes 0..127,
                # cols 256:512 -> nodes 128..255 ----
                cps = cps_pool.tile([128, 2 * N], F32, tag="c",
                                    name=f"cps_{rp}")

                # ---- main loop: 64 outer blocks of 4 nodes ----
                def emit_w2(ob):
                    sil = sils[ob]
                    for hh in range(2):
                        u = 2 * ob + hh
                        nc.tensor.matmul(cps[:], zw[:, 128 - u:256 - u],
                                         sil[:, 512 * hh:512 * hh + 512],
                                         start=(u == 0), stop=(u == 127),
                                         skip_group_check=True)

                sils = {}
                for ob in range(64):
                    pre = pre_pool.tile([128, 1024], F32, tag="pre",
                                        name=f"pre{ob}_{rp}")
                    if variant != "noA":
                        for hh in range(2):
                            out_sl = pre[:, 512 * hh:512 * hh + 512]
                            rhs_a = hT[:, :].unsqueeze(1).broadcast_to(
                                [H, 2, N])
                            nc.tensor.matmul(out_sl, wa[:], rhs_a, start=True,
                                             stop=(variant == "noBd"),
                                             skip_group_check=True)
                    for hh in range(2):
                        if variant == "noBd":
                            break
                        u = 2 * ob + hh
                        out_sl = pre[:, 512 * hh:512 * hh + 512]
                        if variant == "nostage":
                            rhs_bd = stages[0][0:3, 0:512]
                        else:
                            st = stages[u // 4]
                            rhs_bd = st[0:3, (u % 4) * 512:(u % 4) * 512 + 512]
                        nc.tensor.matmul(out_sl,
                                         bwd[0:3, u * H:(u + 1) * H],
                                         rhs_bd, start=(variant == "noA"),
                                         stop=True, skip_group_check=True)
                    sil = lpool.tile([128, 1024], F32R, tag="sil",
                                     name=f"sil{ob}_{rp}")
                    if variant == "dvesilu":
                        nc.vector.tensor_copy(sil[:], pre[:])
                    else:
                        nc.scalar.activation(sil[:], pre[:], Act.Silu,
                                             bias=b1c[:, 0:1])
                    sils[ob] = sil
                    if variant == "noW2":
                        continue
                    # software-pipeline: emit W2 for the PREVIOUS block so PE
                    # has this block's pre-MMs queued while silu(ob-1) runs
                    if ob > 0:
                        emit_w2(ob - 1)
                    if ob == 63:
                        emit_w2(63)

                # ---- final: v = coeff @ pos - rowsum(coeff) * pos_p ----
                for t in range(2):
                    csb = fpool.tile([128, N], F32, tag="csb",
                                     name=f"csb{t}_{rp}")
                    nc.vector.tensor_scalar(csb[:], cps[:, N * t:N * (t + 1)],
                                            b2c[:, 0:1],
                                            None, Alu.add)
                    vcol = fpool.tile([128, 3], F32, tag="vcol",
                                      name=f"vcol{t}_{rp}")
                    scr = fpool.tile([128, N], F32, tag="scr",
                                     name=f"scr{t}_{rp}")
                    for a in range(3):
                        nc.vector.scalar_tensor_tensor(
                            scr[:], csb[:], 1.0, rep[a][:], Alu.mult, Alu.mult,
                            accum_out=vcol[:, a:a + 1])
                    rs = fpool.tile([128, 1], F32, tag="rs",
                                    name=f"rs{t}_{rp}")
                    nc.vector.tensor_scalar(scr[:], csb[:], 1.0, None,
                                            Alu.mult, Alu.add,
                                            accum_out=rs[:, 0:1])
                    rsp = fpool.tile([128, 3], F32, tag="rsp",
                                     name=f"rsp{t}_{rp}")
                    nc.vector.tensor_scalar(rsp[:], pcol[t][:], rs[:, 0:1],
                                            None, Alu.mult)
                    vt = fpool.tile([128, 3], F32, tag="vt",
                                    name=f"vt{t}_{rp}")
                    nc.vector.tensor_tensor(vt[:], vcol[:], rsp[:],
                                            Alu.subtract)
                    nc.sync.dma_start(v_d[128 * t:128 * (t + 1), :], vt[:])

    nc.compile()
    return nc


def _prep_consts(W1, b1, W2, b2):
    wa = np.ascontiguousarray(W1[:H], dtype=np.float32)
    wb = np.ascontiguousarray(W1[H:2 * H], dtype=np.float32)
    wd = W1[2 * H].astype(np.float32)
    wdrep = np.ascontiguousarray(np.tile(wd, 128)[None, :])
    ind = np.zeros((2, 4096), dtype=np.float32)
    cols = np.arange(4096)
    ind[0, (cols % 512) < 256] = 1.0
    ind[1, (cols % 512) >= 256] = 1.0
    zw = np.zeros((H, 2 * H), dtype=np.float32)
    zw[:, H] = W2[:, 0]
    b1c = np.ascontiguousarray(b1.reshape(H, 1), dtype=np.float32)
    b2c = np.full((128, 1), float(np.asarray(b2).reshape(-1)[0]),
                  dtype=np.float32)
    return dict(wa=wa, wb=wb, wdrep=wdrep, ind=ind, zw=zw, b1c=b1c, b2c=b2c)


def _make_in_maps(h, pos, consts):
    in_maps = []
    for g in range(B):
        hg = h[g * N:(g + 1) * N]
        pg = pos[g * N:(g + 1) * N]
        rep3 = np.ascontiguousarray(
            np.broadcast_to(pg.T[:, None, :], (3, 128, N)), dtype=np.float32)
        m = {"hT": np.ascontiguousarray(hg.T), "pos": pg, "rep3": rep3}
        m.update(consts)
        in_maps.append(m)
    return in_maps


def kernel(h, pos, batch, W1, b1, W2, b2, **unused):
    from concourse.bass_utils import run_bass_kernel_spmd

    h = np.ascontiguousarray(np.asarray(h, dtype=np.float32))
    pos = np.ascontiguousarray(np.asarray(pos, dtype=np.float32))
    W1 = np.asarray(W1, dtype=np.float32)
    b1 = np.asarray(b1, dtype=np.float32)
    W2 = np.asarray(W2, dtype=np.float32)
    b2 = np.asarray(b2, dtype=np.float32)

    if "nc" not in _cache:
        _cache["nc"] = _build()
    nc = _cache["nc"]

    consts = _prep_consts(W1, b1, W2, b2)
    in_maps = _make_in_maps(h, pos, consts)
    res = run_bass_kernel_spmd(nc, in_maps, core_ids=list(range(NCORES)))
    return np.concatenate([r["v"] for r in res.results], axis=0)



# revision 2
# speedup vs baseline: 1.8810x; 1.8810x over previous
"""Trainium2 Bass kernel for nn_EquivariantVelocityHead.

Full-input contract: kernel(**inputs) takes the unsharded inputs (as in
setup_inputs()) and returns the full [B*N, 3] output. Internally shards
data-parallel over the graph dimension B across 8 NeuronCores (all pairwise
interactions are intra-graph), with the tiny phi-MLP weights replicated.

Math (per graph, N=256 nodes, H=128):
  A = h @ W1[:H];  Bm = h @ W1[H:2H];  wd = W1[2H];  (phi layer 1 split)
  pre[p,q,:] = A[q] + Bm[p] + dist[p,q]*wd + b1
  coeff[p,q] = silu(pre) @ W2 + b2
  v[p] = sum_q coeff[p,q] * (pos[q] - pos[p])
       = coeff @ pos - rowsum(coeff) * pos[p]

v2 device layout (all matmul operands bf16 so FWL fast-weight-load kicks in):
  Node pairing: u in [0,128) covers nodes (u, u+128); a 512-col block is
  (t2, q) with t2 the pair half. Super-iteration a in [0,32) processes the
  four u's {a, a+32, a+64, a+96} (index i = u//32):
   - A-pass: lhsT=Wa, rhs = hT broadcast [H,2,N], one 512-col MM per u,
     start of the PSUM accumulation for that bank.
   - Bd-pass: K=3 stationary [wd; Bm[u]; Bm[u+128]] at partitions 32i..32i+2
     with moving rows [dist_u; ind_t0; ind_t1]; the four u's go to the four
     PE row groups (tile_position=(32i,0)) so they run concurrently.
   - silu+b1 fused on ScalarE reading PSUM, bf16 out.
   - W2-pass: stationary [128,32] slice of a sliding (zeros|W2|zeros) tile,
     at column group i (tile_position=(0,32i)); node-pair u accumulates into
     cps partition u (= a within group i). Four groups run concurrently.
  Final contraction v = coeff@pos - rowsum(coeff)*pos on VectorE in fp32.
"""
import sys

sys.path.insert(0, "/opt/trn_rl_repo")

import numpy as np

B, N, H = 8, 256, 128
NCORES = 8

_cache = {}


def _build():
    import concourse.bacc as bacc
    import concourse.mybir as mybir
    import concourse.tile as tile

    F32 = mybir.dt.float32
    BF16 = mybir.dt.bfloat16
    Alu = mybir.AluOpType
    Act = mybir.ActivationFunctionType

    nc = bacc.Bacc()

    hT_d = nc.declare_dram_parameter("hT", [H, N], BF16, isOutput=False)
    pos_d = nc.declare_dram_parameter("pos", [N, 3], F32, isOutput=False)
    rep_d = nc.declare_dram_parameter("rep3", [3, 128, N], F32, isOutput=False)
    wa_d = nc.declare_dram_parameter("wa", [H, H], BF16, isOutput=False)
    wb_d = nc.declare_dram_parameter("wb", [H, H], BF16, isOutput=False)
    wdrep_d = nc.declare_dram_parameter("wdrep", [1, 32 * H], BF16,
                                        isOutput=False)
    ind_d = nc.declare_dram_parameter("ind", [2, 32 * 512], BF16,
                                      isOutput=False)
    zws_d = nc.declare_dram_parameter("zws", [H, 63], BF16, isOutput=False)
    b1c_d = nc.declare_dram_parameter("b1c", [H, 1], F32, isOutput=False)
    b2c_d = nc.declare_dram_parameter("b2c", [128, 1], F32, isOutput=False)
    v_d = nc.declare_dram_parameter("v", [N, 3], F32, isOutput=True)

    with tile.TileContext(nc) as tc:
        with (
            tc.tile_pool(name="const", bufs=1) as cpool,
            tc.tile_pool(name="work", bufs=2) as wpool,
            tc.tile_pool(name="silu", bufs=4) as lpool,
            tc.tile_pool(name="fin", bufs=2) as fpool,
            tc.tile_pool(name="pre", bufs=3, space="PSUM") as pre_pool,
            tc.tile_pool(name="cps", bufs=1, space="PSUM") as cps_pool,
            tc.tile_pool(name="bps", bufs=1, space="PSUM") as bps_pool,
        ):
            # ---- constants / inputs ----
            hT = cpool.tile([H, N], BF16, tag="hT")
            nc.sync.dma_start(hT[:], hT_d[:])
            wa = cpool.tile([H, H], BF16, tag="wa")
            nc.sync.dma_start(wa[:], wa_d[:])
            wb = cpool.tile([H, H], BF16, tag="wb")
            nc.sync.dma_start(wb[:], wb_d[:])
            zws = cpool.tile([H, 63], BF16, tag="zws")
            nc.sync.dma_start(zws[:], zws_d[:])
            b1c = cpool.tile([H, 1], F32, tag="b1c")
            nc.sync.dma_start(b1c[:], b1c_d[:])
            b2c = cpool.tile([128, 1], F32, tag="b2c")
            nc.sync.dma_start(b2c[:], b2c_d[:])
            rep = []
            for a in range(3):
                r = cpool.tile([128, N], F32, tag=f"rep{a}")
                nc.sync.dma_start(r[:], rep_d[a])
                rep.append(r)
            pcol = []
            for t in range(2):
                p = cpool.tile([128, 3], F32, tag=f"pcol{t}")
                nc.sync.dma_start(p[:], pos_d[128 * t:128 * (t + 1), :])
                pcol.append(p)

            # Bd stationaries: per u, rows [wd; Bm[u]; Bm[u+128]] live at
            # partitions 32i..32i+2 (i=u//32), column block a=u%32.
            bwd4 = cpool.tile([128, 32 * H], BF16, tag="bwd4")
            # Bd moving rows: [dist_u ; ind_t0 ; ind_t1] same partitions,
            # block a at cols 512a..512a+512 = (t2, q).
            stage = cpool.tile([128, 32 * 512], BF16, tag="stage")
            for i in range(4):
                nc.sync.dma_start(bwd4[32 * i:32 * i + 1, :], wdrep_d[:])
                nc.sync.dma_start(stage[32 * i + 1:32 * i + 3, :], ind_d[:])

            # ---- Bm = h @ Wb, scattered into bwd4 partitions 32i+1,32i+2 ----
            bsb = []
            for t in range(2):
                bp = bps_pool.tile([128, H], F32, tag="bps")
                nc.tensor.matmul(bp[:], hT[:, 128 * t:128 * (t + 1)],
                                 wb[:], start=True, stop=True,
                                 skip_group_check=True)
                bs = wpool.tile([128, H], BF16, tag="bsb", name=f"bsb{t}")
                nc.vector.tensor_copy(bs[:], bp[:])
                bsb.append(bs)
            for t in range(2):
                for i in range(4):
                    dst = bwd4[32 * i + 1 + t:32 * i + 2 + t, :]
                    nc.sync.dma_start(
                        dst.rearrange("o (a c) -> o a c", c=H),
                        bsb[t][32 * i:32 * i + 32, :])

            # ---- dist tiles [p-part, q-free] then staged to partition rows --
            for t in range(2):
                dx = wpool.tile([128, N], F32, tag="dx", name=f"dx{t}")
                dy = wpool.tile([128, N], F32, tag="dy", name=f"dy{t}")
                dz = wpool.tile([128, N], F32, tag="dz", name=f"dz{t}")
                nc.vector.tensor_scalar(dx[:], rep[0][:], pcol[t][:, 0:1],
                                        None, Alu.subtract)
                nc.vector.tensor_scalar(dy[:], rep[1][:], pcol[t][:, 1:2],
                                        None, Alu.subtract)
                nc.vector.tensor_scalar(dz[:], rep[2][:], pcol[t][:, 2:3],
                                        None, Alu.subtract)
                sx = wpool.tile([128, N], F32, tag="sx", name=f"sx{t}")
                sy = wpool.tile([128, N], F32, tag="sy", name=f"sy{t}")
                nc.vector.tensor_tensor(sx[:], dx[:], dx[:], Alu.mult)
                nc.vector.tensor_tensor(sy[:], dy[:], dy[:], Alu.mult)
                nc.vector.tensor_tensor(sx[:], sx[:], sy[:], Alu.add)
                nc.vector.tensor_tensor(sy[:], dz[:], dz[:], Alu.mult)
                nc.vector.tensor_tensor(sx[:], sx[:], sy[:], Alu.add)
                db = wpool.tile([128, N], BF16, tag="db", name=f"db{t}")
                nc.scalar.activation(db[:], sx[:], Act.Sqrt)
                for i in range(4):
                    dst = stage[32 * i:32 * i + 1, :]
                    dst = dst.rearrange("o (a two q) -> o a two q",
                                        two=2, q=N)
                    nc.sync.dma_start(dst[:, :, t, :],
                                      db[32 * i:32 * i + 32, :])

            # ---- coeff accumulator: partition u, cols (t2, q) ----
            cps = cps_pool.tile([128, 2 * N], F32, tag="c")

            rhs_a = hT[:, :].unsqueeze(1).broadcast_to([H, 2, N])

            def emit_w2(b):
                s0, s1 = sils[b]
                for i in range(4):
                    sil = s0 if i < 2 else s1
                    nc.tensor.matmul(cps[32 * i:32 * i + 32, :],
                                     zws[:, 31 - b:63 - b],
                                     sil[:, 512 * (i % 2):512 * (i % 2) + 512],
                                     start=(b == 0), stop=(b == 31),
                                     skip_group_check=True,
                                     tile_position=(0, 32 * i))

            sils = {}
            for a in range(32):
                pres = []
                for half in range(2):
                    pre = pre_pool.tile([128, 1024], F32, tag="pre",
                                        name=f"pre{half}_{a}")
                    for hh in range(2):
                        nc.tensor.matmul(pre[:, 512 * hh:512 * hh + 512],
                                         wa[:], rhs_a, start=True, stop=False,
                                         skip_group_check=True)
                    pres.append(pre)
                for i in range(4):
                    pre = pres[i // 2]
                    out_sl = pre[:, 512 * (i % 2):512 * (i % 2) + 512]
                    nc.tensor.matmul(out_sl,
                                     bwd4[32 * i:32 * i + 3,
                                          H * a:H * a + H],
                                     stage[32 * i:32 * i + 3,
                                           512 * a:512 * a + 512],
                                     start=False, stop=True,
                                     skip_group_check=True,
                                     tile_position=(32 * i, 0))
                cur = []
                for half in range(2):
                    sil = lpool.tile([128, 1024], BF16, tag="sil",
                                     name=f"sil{half}_{a}")
                    nc.scalar.activation(sil[:], pres[half][:], Act.Silu,
                                         bias=b1c[:, 0:1])
                    cur.append(sil)
                sils[a] = cur
                # software-pipeline: emit W2 for the PREVIOUS super-iteration
                if a > 0:
                    emit_w2(a - 1)
                if a == 31:
                    emit_w2(31)

            # ---- final: v = coeff @ pos - rowsum(coeff) * pos_p ----
            for t in range(2):
                csb = fpool.tile([128, N], F32, tag="csb", name=f"csb{t}")
                nc.vector.tensor_scalar(csb[:], cps[:, N * t:N * (t + 1)],
                                        b2c[:, 0:1], None, Alu.add)
                vcol = fpool.tile([128, 3], F32, tag="vcol", name=f"vcol{t}")
                scr = fpool.tile([128, N], F32, tag="scr", name=f"scr{t}")
                for a in range(3):
                    nc.vector.scalar_tensor_tensor(
                        scr[:], csb[:], 1.0, rep[a][:], Alu.mult, Alu.mult,
                        accum_out=vcol[:, a:a + 1])
                rs = fpool.tile([128, 1], F32, tag="rs", name=f"rs{t}")
                nc.vector.tensor_scalar(scr[:], csb[:], 1.0, None,
                                        Alu.mult, Alu.add,
                                        accum_out=rs[:, 0:1])
                rsp = fpool.tile([128, 3], F32, tag="rsp", name=f"rsp{t}")
                nc.vector.tensor_scalar(rsp[:], pcol[t][:], rs[:, 0:1],
                                        None, Alu.mult)
                vt = fpool.tile([128, 3], F32, tag="vt", name=f"vt{t}")
                nc.vector.tensor_tensor(vt[:], vcol[:], rsp[:], Alu.subtract)
                nc.sync.dma_start(v_d[128 * t:128 * (t + 1), :], vt[:])

    nc.compile()
    return nc


def _prep_consts(W1, b1, W2, b2):
    import ml_dtypes
    bf16 = ml_dtypes.bfloat16

    wa = np.ascontiguousarray(W1[:H], dtype=bf16)
    wb = np.ascontiguousarray(W1[H:2 * H], dtype=bf16)
    wd = W1[2 * H].astype(np.float32)
    wdrep = np.ascontiguousarray(np.tile(wd, 32)[None, :]).astype(bf16)
    ind = np.zeros((2, 32 * 512), dtype=np.float32)
    cols = np.arange(32 * 512)
    ind[0, (cols % 512) < 256] = 1.0
    ind[1, (cols % 512) >= 256] = 1.0
    ind = ind.astype(bf16)
    zws = np.zeros((H, 63), dtype=np.float32)
    zws[:, 31] = W2[:, 0]
    zws = zws.astype(bf16)
    b1c = np.ascontiguousarray(b1.reshape(H, 1), dtype=np.float32)
    b2c = np.full((128, 1), float(np.asarray(b2).reshape(-1)[0]),
                  dtype=np.float32)
    return dict(wa=wa, wb=wb, wdrep=wdrep, ind=ind, zws=zws, b1c=b1c, b2c=b2c)


def _make_in_maps(h, pos, consts):
    import ml_dtypes
    bf16 = ml_dtypes.bfloat16

    in_maps = []
    for g in range(B):
        hg = h[g * N:(g + 1) * N]
        pg = pos[g * N:(g + 1) * N]
        rep3 = np.ascontiguousarray(
            np.broadcast_to(pg.T[:, None, :], (3, 128, N)), dtype=np.float32)
        m = {"hT": np.ascontiguousarray(hg.T).astype(bf16), "pos": pg,
             "rep3": rep3}
        m.update(consts)
        in_maps.append(m)
    return in_maps


def kernel(h, pos, batch, W1, b1, W2, b2, **unused):
    from concourse.bass_utils import run_bass_kernel_spmd

    h = np.ascontiguousarray(np.asarray(h, dtype=np.float32))
    pos = np.ascontiguousarray(np.asarray(pos, dtype=np.float32))
    W1 = np.asarray(W1, dtype=np.float32)
    b1 = np.asarray(b1, dtype=np.float32)
    W2 = np.asarray(W2, dtype=np.float32)
    b2 = np.asarray(b2, dtype=np.float32)

    if "nc" not in _cache:
        _cache["nc"] = _build()
    nc = _cache["nc"]

    consts = _prep_consts(W1, b1, W2, b2)
    in_maps = _make_in_maps(h, pos, consts)
    res = run_bass_kernel_spmd(nc, in_maps, core_ids=list(range(NCORES)))
    return np.concatenate([r["v"] for r in res.results], axis=0)


# revision 7
# speedup vs baseline: 1.8945x; 1.0071x over previous
"""Trainium2 Bass kernel for nn_EquivariantVelocityHead.

Full-input contract: kernel(**inputs) takes the unsharded inputs (as in
setup_inputs()) and returns the full [B*N, 3] output. Internally shards
data-parallel over the graph dimension B across 8 NeuronCores (all pairwise
interactions are intra-graph), with the tiny phi-MLP weights replicated.

Math (per graph, N=256 nodes, H=128):
  A = h @ W1[:H];  Bm = h @ W1[H:2H];  wd = W1[2H];  (phi layer 1 split)
  pre[p,q,:] = A[q] + Bm[p] + dist[p,q]*wd + b1
  coeff[p,q] = silu(pre) @ W2 + b2
  v[p] = sum_q coeff[p,q] * (pos[q] - pos[p])
       = coeff @ pos - rowsum(coeff) * pos[p]

v2 device layout (all matmul operands bf16 so FWL fast-weight-load kicks in):
  Node pairing: u in [0,128) covers nodes (u, u+128); a 512-col block is
  (t2, q) with t2 the pair half. Super-iteration a in [0,32) processes the
  four u's {a, a+32, a+64, a+96} (index i = u//32):
   - A-pass: lhsT=Wa, rhs = hT broadcast [H,2,N], one 512-col MM per u,
     start of the PSUM accumulation for that bank.
   - Bd-pass: K=3 stationary [wd; Bm[u]; Bm[u+128]] at partitions 32i..32i+2
     with moving rows [dist_u; ind_t0; ind_t1]; the four u's go to the four
     PE row groups (tile_position=(32i,0)) so they run concurrently.
   - silu+b1 fused on ScalarE reading PSUM, bf16 out.
   - W2-pass: stationary [128,32] slice of a sliding (zeros|W2|zeros) tile,
     at column group i (tile_position=(0,32i)); node-pair u accumulates into
     cps partition u (= a within group i). Four groups run concurrently.
  Final contraction v = coeff@pos - rowsum(coeff)*pos on VectorE in fp32.
"""
import sys

sys.path.insert(0, "/opt/trn_rl_repo")

import numpy as np

B, N, H = 8, 256, 128
NCORES = 8

_cache = {}


def _build():
    import concourse.bacc as bacc
    import concourse.mybir as mybir
    import concourse.tile as tile

    F32 = mybir.dt.float32
    BF16 = mybir.dt.bfloat16
    Alu = mybir.AluOpType
    Act = mybir.ActivationFunctionType

    nc = bacc.Bacc()

    hT_d = nc.declare_dram_parameter("hT", [H, N], BF16, isOutput=False)
    pos_d = nc.declare_dram_parameter("pos", [N, 3], F32, isOutput=False)
    rep_d = nc.declare_dram_parameter("rep3", [128, 3 * N], F32,
                                      isOutput=False)
    wa_d = nc.declare_dram_parameter("wa", [H, H], BF16, isOutput=False)
    wb_d = nc.declare_dram_parameter("wb", [H, H], BF16, isOutput=False)
    wdrep_d = nc.declare_dram_parameter("wdrep", [1, 32 * H], BF16,
                                        isOutput=False)
    ind_d = nc.declare_dram_parameter("ind", [2, 32 * 512], BF16,
                                      isOutput=False)
    zws_d = nc.declare_dram_parameter("zws", [H, 63], BF16, isOutput=False)
    b1c_d = nc.declare_dram_parameter("b1c", [H, 1], F32, isOutput=False)
    corr3_d = nc.declare_dram_parameter("corr3", [128, 3], F32,
                                        isOutput=False)
    b256_d = nc.declare_dram_parameter("b256", [128, 1], F32, isOutput=False)
    v_d = nc.declare_dram_parameter("v", [N, 3], F32, isOutput=True)

    with tile.TileContext(nc) as tc:
        with (
            tc.tile_pool(name="const", bufs=1) as cpool,
            tc.tile_pool(name="work", bufs=2) as wpool,
            tc.tile_pool(name="silu", bufs=6) as lpool,
            tc.tile_pool(name="fin", bufs=2) as fpool,
            tc.tile_pool(name="pre", bufs=3, space="PSUM") as pre_pool,
            tc.tile_pool(name="cps", bufs=1, space="PSUM") as cps_pool,
            tc.tile_pool(name="bps", bufs=1, space="PSUM") as bps_pool,
        ):
            # ---- constants / inputs ----
            # SP queue: critical path first (rep/pcol feed the dist pipeline,
            # hT/wb feed the Bm matmuls).
            rep_all = cpool.tile([128, 3 * N], F32, tag="rep_all")
            nc.sync.dma_start(rep_all[:], rep_d[:])
            rep = [rep_all[:, N * a:N * (a + 1)] for a in range(3)]
            pcall = cpool.tile([128, 6], F32, tag="pcall")
            nc.sync.dma_start(pcall.rearrange("p (t c) -> p t c", t=2),
                              pos_d.rearrange("(t p) c -> p t c", t=2))
            pcol = [pcall[:, 3 * t:3 * t + 3] for t in range(2)]
            hT = cpool.tile([H, N], BF16, tag="hT")
            nc.sync.dma_start(hT[:], hT_d[:])
            wb = cpool.tile([H, H], BF16, tag="wb")
            nc.sync.dma_start(wb[:], wb_d[:])
            wa = cpool.tile([H, H], BF16, tag="wa")
            nc.sync.dma_start(wa[:], wa_d[:])

            # ACT queue (idle in prologue): Bd stationary/moving constants.
            # Bd stationaries: per u, rows [wd; Bm[u]; Bm[u+128]] live at
            # partitions 32i..32i+2 (i=u//32), column block a=u%32.
            bwd4 = cpool.tile([128, 32 * H], BF16, tag="bwd4")
            # Bd moving rows: [dist_u ; ind_t0 ; ind_t1] same partitions,
            # block a at cols 512a..512a+512 = (t2, q).
            stage = cpool.tile([128, 32 * 512], BF16, tag="stage")
            for i in range(4):
                nc.scalar.dma_start(bwd4[32 * i:32 * i + 1, :], wdrep_d[:])
                nc.scalar.dma_start(stage[32 * i + 1:32 * i + 3, :], ind_d[:])
            zws = cpool.tile([H, 63], BF16, tag="zws")
            nc.scalar.dma_start(zws[:], zws_d[:])
            b1c = cpool.tile([H, 1], F32, tag="b1c")
            nc.scalar.dma_start(b1c[:], b1c_d[:])
            corr3 = cpool.tile([128, 3], F32, tag="corr3")
            nc.scalar.dma_start(corr3[:], corr3_d[:])
            b256 = cpool.tile([128, 1], F32, tag="b256")
            nc.scalar.dma_start(b256[:], b256_d[:])

            # ---- dist: both halves in one [128, 512] set of tiles ----
            dx = wpool.tile([128, 512], F32, tag="dx")
            dy = wpool.tile([128, 512], F32, tag="dy")
            dz = wpool.tile([128, 512], F32, tag="dz")
            for t in range(2):
                nc.vector.tensor_scalar(dx[:, 256 * t:256 * t + 256],
                                        rep[0], pcol[t][:, 0:1],
                                        None, Alu.subtract)
                nc.vector.tensor_scalar(dy[:, 256 * t:256 * t + 256],
                                        rep[1], pcol[t][:, 1:2],
                                        None, Alu.subtract)
                nc.vector.tensor_scalar(dz[:, 256 * t:256 * t + 256],
                                        rep[2], pcol[t][:, 2:3],
                                        None, Alu.subtract)
            sx = wpool.tile([128, 512], F32, tag="sx")
            sy = wpool.tile([128, 512], F32, tag="sy")
            nc.vector.tensor_tensor(sx[:], dx[:], dx[:], Alu.mult)
            nc.vector.tensor_tensor(sy[:], dy[:], dy[:], Alu.mult)
            nc.vector.tensor_tensor(sx[:], sx[:], sy[:], Alu.add)
            nc.vector.tensor_tensor(sy[:], dz[:], dz[:], Alu.mult)
            nc.vector.tensor_tensor(sx[:], sx[:], sy[:], Alu.add)
            db = wpool.tile([128, 512], BF16, tag="db")
            nc.scalar.activation(db[:], sx[:], Act.Sqrt)
            for i in range(4):
                dst = stage[32 * i:32 * i + 1, :]
                dst = dst.rearrange("o (a two q) -> o a two q", two=2, q=N)
                nc.sync.dma_start(dst[:],
                                  db[32 * i:32 * i + 32, :].rearrange(
                                      "a (two q) -> a two q", two=2))

            # ---- Bm = h @ Wb, scattered into bwd4 partitions 32i+1,32i+2 ----
            bsb = []
            for t in range(2):
                bp = bps_pool.tile([128, H], F32, tag="bps")
                nc.tensor.matmul(bp[:], hT[:, 128 * t:128 * (t + 1)],
                                 wb[:], start=True, stop=True,
                                 skip_group_check=True)
                bs = wpool.tile([128, H], BF16, tag="bsb", name=f"bsb{t}")
                nc.vector.tensor_copy(bs[:], bp[:])
                bsb.append(bs)
            for t in range(2):
                for i in range(4):
                    dst = bwd4[32 * i + 1 + t:32 * i + 2 + t, :]
                    nc.sync.dma_start(
                        dst.rearrange("o (a c) -> o a c", c=H),
                        bsb[t][32 * i:32 * i + 32, :])

            # ---- coeff accumulator: partition u, cols (t2, q) ----
            cps = cps_pool.tile([128, 2 * N], F32, tag="c")

            rhs_a = hT[:, :].unsqueeze(1).broadcast_to([H, 2, N])

            def emit_w2(b):
                s0, s1 = sils[b]
                for i in range(4):
                    sil = s0 if i < 2 else s1
                    nc.tensor.matmul(cps[32 * i:32 * i + 32, :],
                                     zws[:, 31 - b:63 - b],
                                     sil[:, 512 * (i % 2):512 * (i % 2) + 512],
                                     start=(b == 0), stop=(b == 31),
                                     skip_group_check=True,
                                     tile_position=(0, 32 * i))

            sils = {}
            for a in range(32):
                pres = []
                for half in range(2):
                    pre = pre_pool.tile([128, 1024], F32, tag="pre",
                                        name=f"pre{half}_{a}")
                    for hh in range(2):
                        nc.tensor.matmul(pre[:, 512 * hh:512 * hh + 512],
                                         wa[:], rhs_a, start=True, stop=False,
                                         skip_group_check=True)
                    pres.append(pre)
                for i in range(4):
                    pre = pres[i // 2]
                    out_sl = pre[:, 512 * (i % 2):512 * (i % 2) + 512]
                    nc.tensor.matmul(out_sl,
                                     bwd4[32 * i:32 * i + 3,
                                          H * a:H * a + H],
                                     stage[32 * i:32 * i + 3,
                                           512 * a:512 * a + 512],
                                     start=False, stop=True,
                                     skip_group_check=True,
                                     tile_position=(32 * i, 0))
                cur = []
                for half in range(2):
                    sil = lpool.tile([128, 1024], BF16, tag="sil",
                                     name=f"sil{half}_{a}")
                    nc.scalar.activation(sil[:], pres[half][:], Act.Silu,
                                         bias=b1c[:, 0:1])
                    cur.append(sil)
                sils[a] = cur
                # software-pipeline: emit W2 for the PREVIOUS super-iteration
                if a > 0:
                    emit_w2(a - 1)
                if a == 31:
                    emit_w2(31)

            # ---- final: v = coeff @ pos - rowsum(coeff) * pos_p, with the
            # +b2 fold done via host-precomputed corr3 = b2*sum_q(pos) and
            # b256 = 256*b2: v = (cps@pos + corr3) - (rowsum(cps)+b256)*pos_p
            for t in range(2):
                cslice = cps[:, N * t:N * (t + 1)]
                vcol = fpool.tile([128, 3], F32, tag="vcol", name=f"vcol{t}")
                scr = fpool.tile([128, N], F32, tag="scr", name=f"scr{t}")
                for a in range(3):
                    nc.vector.scalar_tensor_tensor(
                        scr[:], cslice, 1.0, rep[a], Alu.mult, Alu.mult,
                        accum_out=vcol[:, a:a + 1])
                rs = fpool.tile([128, 1], F32, tag="rs", name=f"rs{t}")
                nc.vector.tensor_scalar(scr[:], cslice, 1.0, None,
                                        Alu.mult, Alu.add,
                                        accum_out=rs[:, 0:1])
                rs2 = fpool.tile([128, 1], F32, tag="rs2", name=f"rs2{t}")
                nc.vector.tensor_tensor(rs2[:], rs[:], b256[:], Alu.add)
                rsp = fpool.tile([128, 3], F32, tag="rsp", name=f"rsp{t}")
                nc.vector.tensor_scalar(rsp[:], pcol[t], rs2[:, 0:1],
                                        None, Alu.mult)
                vt = fpool.tile([128, 3], F32, tag="vt", name=f"vt{t}")
                nc.vector.tensor_tensor(vt[:], vcol[:], corr3[:], Alu.add)
                nc.vector.tensor_tensor(vt[:], vt[:], rsp[:], Alu.subtract)
                nc.sync.dma_start(v_d[128 * t:128 * (t + 1), :], vt[:])

    nc.compile()
    return nc


def _prep_consts(W1, b1, W2, b2):
    import ml_dtypes
    bf16 = ml_dtypes.bfloat16

    wa = np.ascontiguousarray(W1[:H], dtype=bf16)
    wb = np.ascontiguousarray(W1[H:2 * H], dtype=bf16)
    wd = W1[2 * H].astype(np.float32)
    wdrep = np.ascontiguousarray(np.tile(wd, 32)[None, :]).astype(bf16)
    ind = np.zeros((2, 32 * 512), dtype=np.float32)
    cols = np.arange(32 * 512)
    ind[0, (cols % 512) < 256] = 1.0
    ind[1, (cols % 512) >= 256] = 1.0
    ind = ind.astype(bf16)
    zws = np.zeros((H, 63), dtype=np.float32)
    zws[:, 31] = W2[:, 0]
    zws = zws.astype(bf16)
    b1c = np.ascontiguousarray(b1.reshape(H, 1), dtype=np.float32)
    b2v = float(np.asarray(b2).reshape(-1)[0])
    b256 = np.full((128, 1), 256.0 * b2v, dtype=np.float32)
    return dict(wa=wa, wb=wb, wdrep=wdrep, ind=ind, zws=zws, b1c=b1c,
                b256=b256, b2v=b2v)


def _make_in_maps(h, pos, consts):
    import ml_dtypes
    bf16 = ml_dtypes.bfloat16

    consts = dict(consts)
    b2v = consts.pop("b2v")
    in_maps = []
    for g in range(B):
        hg = h[g * N:(g + 1) * N]
        pg = pos[g * N:(g + 1) * N]
        rep3 = np.ascontiguousarray(
            np.broadcast_to(pg.T.reshape(1, 3 * N), (128, 3 * N)),
            dtype=np.float32)
        corr3 = np.ascontiguousarray(
            np.broadcast_to((b2v * pg.sum(axis=0))[None, :], (128, 3)),
            dtype=np.float32)
        m = {"hT": np.ascontiguousarray(hg.T).astype(bf16), "pos": pg,
             "rep3": rep3, "corr3": corr3}
        m.update(consts)
        in_maps.append(m)
    return in_maps


def kernel(h, pos, batch, W1, b1, W2, b2, **unused):
    from concourse.bass_utils import run_bass_kernel_spmd

    h = np.ascontiguousarray(np.asarray(h, dtype=np.float32))
    pos = np.ascontiguousarray(np.asarray(pos, dtype=np.float32))
    W1 = np.asarray(W1, dtype=np.float32)
    b1 = np.asarray(b1, dtype=np.float32)
    W2 = np.asarray(W2, dtype=np.float32)
    b2 = np.asarray(b2, dtype=np.float32)

    if "nc" not in _cache:
        _cache["nc"] = _build()
    nc = _cache["nc"]

    consts = _prep_consts(W1, b1, W2, b2)
    in_maps = _make_in_maps(h, pos, consts)
    res = run_bass_kernel_spmd(nc, in_maps, core_ids=list(range(NCORES)))
    return np.concatenate([r["v"] for r in res.results], axis=0)


# revision 9
# speedup vs baseline: 1.9327x; 1.0202x over previous
"""Trainium2 Bass kernel for nn_EquivariantVelocityHead.

Full-input contract: kernel(**inputs) takes the unsharded inputs (as in
setup_inputs()) and returns the full [B*N, 3] output. Internally shards
data-parallel over the graph dimension B across 8 NeuronCores (all pairwise
interactions are intra-graph), with the tiny phi-MLP weights replicated.

Math (per graph, N=256 nodes, H=128):
  A = h @ W1[:H];  Bm = h @ W1[H:2H];  wd = W1[2H];  (phi layer 1 split)
  pre[p,q,:] = A[q] + Bm[p] + dist[p,q]*wd + b1
  coeff[p,q] = silu(pre) @ W2 + b2
  v[p] = sum_q coeff[p,q] * (pos[q] - pos[p])
       = coeff @ pos - rowsum(coeff) * pos[p]

v2 device layout (all matmul operands bf16 so FWL fast-weight-load kicks in):
  Node pairing: u in [0,128) covers nodes (u, u+128); a 512-col block is
  (t2, q) with t2 the pair half. Super-iteration a in [0,32) processes the
  four u's {a, a+32, a+64, a+96} (index i = u//32):
   - A-pass: lhsT=Wa, rhs = hT broadcast [H,2,N], one 512-col MM per u,
     start of the PSUM accumulation for that bank.
   - Bd-pass: K=3 stationary [wd; Bm[u]; Bm[u+128]] at partitions 32i..32i+2
     with moving rows [dist_u; ind_t0; ind_t1]; the four u's go to the four
     PE row groups (tile_position=(32i,0)) so they run concurrently.
   - silu+b1 fused on ScalarE reading PSUM, bf16 out.
   - W2-pass: stationary [128,32] slice of a sliding (zeros|W2|zeros) tile,
     at column group i (tile_position=(0,32i)); node-pair u accumulates into
     cps partition u (= a within group i). Four groups run concurrently.
  Final contraction v = coeff@pos - rowsum(coeff)*pos on VectorE in fp32.
"""
import sys

sys.path.insert(0, "/opt/trn_rl_repo")

import numpy as np

B, N, H = 8, 256, 128
NCORES = 8

_cache = {}


def _build():
    import concourse.bacc as bacc
    import concourse.mybir as mybir
    import concourse.tile as tile

    F32 = mybir.dt.float32
    BF16 = mybir.dt.bfloat16
    Alu = mybir.AluOpType
    Act = mybir.ActivationFunctionType

    nc = bacc.Bacc()

    hT_d = nc.declare_dram_parameter("hT", [H, N], BF16, isOutput=False)
    pos_d = nc.declare_dram_parameter("pos", [N, 3], F32, isOutput=False)
    rep_d = nc.declare_dram_parameter("rep3", [128, 3 * N], F32,
                                      isOutput=False)
    wa_d = nc.declare_dram_parameter("wa", [H, H], BF16, isOutput=False)
    wb_d = nc.declare_dram_parameter("wb", [H, H], BF16, isOutput=False)
    wdrep_d = nc.declare_dram_parameter("wdrep", [1, 32 * H], BF16,
                                        isOutput=False)
    ind_d = nc.declare_dram_parameter("ind", [2, 32 * 512], BF16,
                                      isOutput=False)
    zws_d = nc.declare_dram_parameter("zws", [H, 63], BF16, isOutput=False)
    b1c_d = nc.declare_dram_parameter("b1c", [H, 1], F32, isOutput=False)
    corr3_d = nc.declare_dram_parameter("corr3", [128, 3], F32,
                                        isOutput=False)
    b256_d = nc.declare_dram_parameter("b256", [128, 1], F32, isOutput=False)
    v_d = nc.declare_dram_parameter("v", [N, 3], F32, isOutput=True)

    with tile.TileContext(nc) as tc:
        with (
            tc.tile_pool(name="const", bufs=1) as cpool,
            tc.tile_pool(name="work", bufs=2) as wpool,
            tc.tile_pool(name="silu", bufs=6) as lpool,
            tc.tile_pool(name="fin", bufs=2) as fpool,
            tc.tile_pool(name="pre", bufs=3, space="PSUM") as pre_pool,
            tc.tile_pool(name="cps", bufs=1, space="PSUM") as cps_pool,
            tc.tile_pool(name="bps", bufs=1, space="PSUM") as bps_pool,
        ):
            # ---- constants / inputs ----
            # SP queue: dist-critical path first (rep/pcol feed the dist
            # pipeline). hT/wb/wa go on the ACT queue, which is idle until
            # the sqrt, so both queues issue prologue DMAs in parallel.
            rep_all = cpool.tile([128, 3 * N], F32, tag="rep_all")
            nc.sync.dma_start(rep_all[:], rep_d[:])
            rep = [rep_all[:, N * a:N * (a + 1)] for a in range(3)]
            pcall = cpool.tile([128, 6], F32, tag="pcall")
            nc.sync.dma_start(pcall.rearrange("p (t c) -> p t c", t=2),
                              pos_d.rearrange("(t p) c -> p t c", t=2))
            pcol = [pcall[:, 3 * t:3 * t + 3] for t in range(2)]
            hT = cpool.tile([H, N], BF16, tag="hT")
            nc.scalar.dma_start(hT[:], hT_d[:])
            wb = cpool.tile([H, H], BF16, tag="wb")
            nc.scalar.dma_start(wb[:], wb_d[:])
            wa = cpool.tile([H, H], BF16, tag="wa")
            nc.scalar.dma_start(wa[:], wa_d[:])

            # Bd stationaries: per u, rows [wd; Bm[u]; Bm[u+128]] live at
            # partitions 32i..32i+2 (i=u//32), column block a=u%32.
            bwd4 = cpool.tile([128, 32 * H], BF16, tag="bwd4")
            # Bd moving rows: [dist_u ; ind_t0 ; ind_t1] same partitions,
            # block a at cols 512a..512a+512 = (t2, q).
            stage = cpool.tile([128, 32 * 512], BF16, tag="stage")
            for i in range(4):
                nc.sync.dma_start(bwd4[32 * i:32 * i + 1, :], wdrep_d[:])
                nc.sync.dma_start(stage[32 * i + 1:32 * i + 3, :], ind_d[:])

            # ---- dist: both halves in one [128, 512] set of tiles ----
            dx = wpool.tile([128, 512], F32, tag="dx")
            dy = wpool.tile([128, 512], F32, tag="dy")
            dz = wpool.tile([128, 512], F32, tag="dz")
            for t in range(2):
                nc.vector.tensor_scalar(dx[:, 256 * t:256 * t + 256],
                                        rep[0], pcol[t][:, 0:1],
                                        None, Alu.subtract)
                nc.vector.tensor_scalar(dy[:, 256 * t:256 * t + 256],
                                        rep[1], pcol[t][:, 1:2],
                                        None, Alu.subtract)
                nc.vector.tensor_scalar(dz[:, 256 * t:256 * t + 256],
                                        rep[2], pcol[t][:, 2:3],
                                        None, Alu.subtract)
            sx = wpool.tile([128, 512], F32, tag="sx")
            sy = wpool.tile([128, 512], F32, tag="sy")
            nc.vector.tensor_tensor(sx[:], dx[:], dx[:], Alu.mult)
            nc.vector.tensor_tensor(sy[:], dy[:], dy[:], Alu.mult)
            nc.vector.tensor_tensor(sx[:], sx[:], sy[:], Alu.add)
            nc.vector.tensor_tensor(sy[:], dz[:], dz[:], Alu.mult)
            nc.vector.tensor_tensor(sx[:], sx[:], sy[:], Alu.add)
            db = wpool.tile([128, 512], BF16, tag="db")
            nc.scalar.activation(db[:], sx[:], Act.Sqrt)

            # remaining small consts: ACT queue, after the sqrt
            zws = cpool.tile([H, 63], BF16, tag="zws")
            nc.scalar.dma_start(zws[:], zws_d[:])
            b1c = cpool.tile([H, 1], F32, tag="b1c")
            nc.scalar.dma_start(b1c[:], b1c_d[:])
            corr3 = cpool.tile([128, 3], F32, tag="corr3")
            nc.scalar.dma_start(corr3[:], corr3_d[:])
            b256 = cpool.tile([128, 1], F32, tag="b256")
            nc.scalar.dma_start(b256[:], b256_d[:])

            # ---- Bm = h @ Wb, scattered into bwd4 partitions 32i+1,32i+2 ----
            bsb = []
            for t in range(2):
                bp = bps_pool.tile([128, H], F32, tag="bps")
                nc.tensor.matmul(bp[:], hT[:, 128 * t:128 * (t + 1)],
                                 wb[:], start=True, stop=True,
                                 skip_group_check=True)
                bs = wpool.tile([128, H], BF16, tag="bsb", name=f"bsb{t}")
                nc.vector.tensor_copy(bs[:], bp[:])
                bsb.append(bs)
            for t in range(2):
                for i in range(4):
                    dst = bwd4[32 * i + 1 + t:32 * i + 2 + t, :]
                    nc.sync.dma_start(
                        dst.rearrange("o (a c) -> o a c", c=H),
                        bsb[t][32 * i:32 * i + 32, :])
            for i in range(4):
                dst = stage[32 * i:32 * i + 1, :]
                dst = dst.rearrange("o (a two q) -> o a two q", two=2, q=N)
                nc.sync.dma_start(dst[:],
                                  db[32 * i:32 * i + 32, :].rearrange(
                                      "a (two q) -> a two q", two=2))

            # ---- coeff accumulator: partition u, cols (t2, q) ----
            cps = cps_pool.tile([128, 2 * N], F32, tag="c")

            rhs_a = hT[:, :].unsqueeze(1).broadcast_to([H, 2, N])

            def emit_w2(b):
                s0, s1 = sils[b]
                for i in range(4):
                    sil = s0 if i < 2 else s1
                    nc.tensor.matmul(cps[32 * i:32 * i + 32, :],
                                     zws[:, 31 - b:63 - b],
                                     sil[:, 512 * (i % 2):512 * (i % 2) + 512],
                                     start=(b == 0), stop=(b == 31),
                                     skip_group_check=True,
                                     tile_position=(0, 32 * i))

            sils = {}
            for a in range(32):
                pres = []
                for half in range(2):
                    pre = pre_pool.tile([128, 1024], F32, tag="pre",
                                        name=f"pre{half}_{a}")
                    for hh in range(2):
                        nc.tensor.matmul(pre[:, 512 * hh:512 * hh + 512],
                                         wa[:], rhs_a, start=True, stop=False,
                                         skip_group_check=True)
                    pres.append(pre)
                for i in range(4):
                    pre = pres[i // 2]
                    out_sl = pre[:, 512 * (i % 2):512 * (i % 2) + 512]
                    nc.tensor.matmul(out_sl,
                                     bwd4[32 * i:32 * i + 3,
                                          H * a:H * a + H],
                                     stage[32 * i:32 * i + 3,
                                           512 * a:512 * a + 512],
                                     start=False, stop=True,
                                     skip_group_check=True,
                                     tile_position=(32 * i, 0))
                cur = []
                for half in range(2):
                    sil = lpool.tile([128, 1024], BF16, tag="sil",
                                     name=f"sil{half}_{a}")
                    nc.scalar.activation(sil[:], pres[half][:], Act.Silu,
                                         bias=b1c[:, 0:1])
                    cur.append(sil)
                sils[a] = cur
                # software-pipeline: emit W2 for the PREVIOUS super-iteration
                if a > 0:
                    emit_w2(a - 1)
                if a == 31:
                    emit_w2(31)

            # ---- final: v = coeff @ pos - rowsum(coeff) * pos_p, with the
            # +b2 fold done via host-precomputed corr3 = b2*sum_q(pos) and
            # b256 = 256*b2: v = (cps@pos + corr3) - (rowsum(cps)+b256)*pos_p
            for t in range(2):
                cslice = cps[:, N * t:N * (t + 1)]
                vcol = fpool.tile([128, 3], F32, tag="vcol", name=f"vcol{t}")
                scr = fpool.tile([128, N], F32, tag="scr", name=f"scr{t}")
                for a in range(3):
                    nc.vector.scalar_tensor_tensor(
                        scr[:], cslice, 1.0, rep[a], Alu.mult, Alu.mult,
                        accum_out=vcol[:, a:a + 1])
                rs = fpool.tile([128, 1], F32, tag="rs", name=f"rs{t}")
                nc.vector.tensor_scalar(scr[:], cslice, 1.0, None,
                                        Alu.mult, Alu.add,
                                        accum_out=rs[:, 0:1])
                rs2 = fpool.tile([128, 1], F32, tag="rs2", name=f"rs2{t}")
                nc.vector.tensor_tensor(rs2[:], rs[:], b256[:], Alu.add)
                rsp = fpool.tile([128, 3], F32, tag="rsp", name=f"rsp{t}")
                nc.vector.tensor_scalar(rsp[:], pcol[t], rs2[:, 0:1],
                                        None, Alu.mult)
                vt = fpool.tile([128, 3], F32, tag="vt", name=f"vt{t}")
                nc.vector.tensor_tensor(vt[:], vcol[:], corr3[:], Alu.add)
                nc.vector.tensor_tensor(vt[:], vt[:], rsp[:], Alu.subtract)
                nc.sync.dma_start(v_d[128 * t:128 * (t + 1), :], vt[:])

    nc.compile()
    return nc


def _prep_consts(W1, b1, W2, b2):
    import ml_dtypes
    bf16 = ml_dtypes.bfloat16

    wa = np.ascontiguousarray(W1[:H], dtype=bf16)
    wb = np.ascontiguousarray(W1[H:2 * H], dtype=bf16)
    wd = W1[2 * H].astype(np.float32)
    wdrep = np.ascontiguousarray(np.tile(wd, 32)[None, :]).astype(bf16)
    ind = np.zeros((2, 32 * 512), dtype=np.float32)
    cols = np.arange(32 * 512)
    ind[0, (cols % 512) < 256] = 1.0
    ind[1, (cols % 512) >= 256] = 1.0
    ind = ind.astype(bf16)
    zws = np.zeros((H, 63), dtype=np.float32)
    zws[:, 31] = W2[:, 0]
    zws = zws.astype(bf16)
    b1c = np.ascontiguousarray(b1.reshape(H, 1), dtype=np.float32)
    b2v = float(np.asarray(b2).reshape(-1)[0])
    b256 = np.full((128, 1), 256.0 * b2v, dtype=np.float32)
    return dict(wa=wa, wb=wb, wdrep=wdrep, ind=ind, zws=zws, b1c=b1c,
                b256=b256, b2v=b2v)


def _make_in_maps(h, pos, consts):
    import ml_dtypes
    bf16 = ml_dtypes.bfloat16

    consts = dict(consts)
    b2v = consts.pop("b2v")
    in_maps = []
    for g in range(B):
        hg = h[g * N:(g + 1) * N]
        pg = pos[g * N:(g + 1) * N]
        rep3 = np.ascontiguousarray(
            np.broadcast_to(pg.T.reshape(1, 3 * N), (128, 3 * N)),
            dtype=np.float32)
        corr3 = np.ascontiguousarray(
            np.broadcast_to((b2v * pg.sum(axis=0))[None, :], (128, 3)),
            dtype=np.float32)
        m = {"hT": np.ascontiguousarray(hg.T).astype(bf16), "pos": pg,
             "rep3": rep3, "corr3": corr3}
        m.update(consts)
        in_maps.append(m)
    return in_maps


def kernel(h, pos, batch, W1, b1, W2, b2, **unused):
    from concourse.bass_utils import run_bass_kernel_spmd

    h = np.ascontiguousarray(np.asarray(h, dtype=np.float32))
    pos = np.ascontiguousarray(np.asarray(pos, dtype=np.float32))
    W1 = np.asarray(W1, dtype=np.float32)
    b1 = np.asarray(b1, dtype=np.float32)
    W2 = np.asarray(W2, dtype=np.float32)
    b2 = np.asarray(b2, dtype=np.float32)

    if "nc" not in _cache:
        _cache["nc"] = _build()
    nc = _cache["nc"]

    consts = _prep_consts(W1, b1, W2, b2)
    in_maps = _make_in_maps(h, pos, consts)
    res = run_bass_kernel_spmd(nc, in_maps, core_ids=list(range(NCORES)))
    return np.concatenate([r["v"] for r in res.results], axis=0)


# revision 13
# speedup vs baseline: 1.9356x; 1.0015x over previous
"""Trainium2 Bass kernel for nn_EquivariantVelocityHead.

Full-input contract: kernel(**inputs) takes the unsharded inputs (as in
setup_inputs()) and returns the full [B*N, 3] output. Internally shards
data-parallel over the graph dimension B across 8 NeuronCores (all pairwise
interactions are intra-graph), with the tiny phi-MLP weights replicated.

Math (per graph, N=256 nodes, H=128):
  A = h @ W1[:H];  Bm = h @ W1[H:2H];  wd = W1[2H];  (phi layer 1 split)
  pre[p,q,:] = A[q] + Bm[p] + dist[p,q]*wd + b1
  coeff[p,q] = silu(pre) @ W2 + b2
  v[p] = sum_q coeff[p,q] * (pos[q] - pos[p])
       = coeff @ pos - rowsum(coeff) * pos[p]

v2 device layout (all matmul operands bf16 so FWL fast-weight-load kicks in):
  Node pairing: u in [0,128) covers nodes (u, u+128); a 512-col block is
  (t2, q) with t2 the pair half. Super-iteration a in [0,32) processes the
  four u's {a, a+32, a+64, a+96} (index i = u//32):
   - A-pass: lhsT=Wa, rhs = hT broadcast [H,2,N], one 512-col MM per u,
     start of the PSUM accumulation for that bank.
   - Bd-pass: K=3 stationary [wd; Bm[u]; Bm[u+128]] at partitions 32i..32i+2
     with moving rows [dist_u; ind_t0; ind_t1]; the four u's go to the four
     PE row groups (tile_position=(32i,0)) so they run concurrently.
   - silu+b1 fused on ScalarE reading PSUM, bf16 out.
   - W2-pass: stationary [128,32] slice of a sliding (zeros|W2|zeros) tile,
     at column group i (tile_position=(0,32i)); node-pair u accumulates into
     cps partition u (= a within group i). Four groups run concurrently.
  Final contraction v = coeff@pos - rowsum(coeff)*pos on VectorE in fp32.
"""
import sys

sys.path.insert(0, "/opt/trn_rl_repo")

import numpy as np

B, N, H = 8, 256, 128
NCORES = 8

_cache = {}


def _build():
    import concourse.bacc as bacc
    import concourse.mybir as mybir
    import concourse.tile as tile

    F32 = mybir.dt.float32
    BF16 = mybir.dt.bfloat16
    Alu = mybir.AluOpType
    Act = mybir.ActivationFunctionType

    nc = bacc.Bacc()

    hT_d = nc.declare_dram_parameter("hT", [H, N], BF16, isOutput=False)
    pos_d = nc.declare_dram_parameter("pos", [N, 3], F32, isOutput=False)
    rep_d = nc.declare_dram_parameter("rep3", [128, 3 * N], F32,
                                      isOutput=False)
    wa_d = nc.declare_dram_parameter("wa", [H, H], BF16, isOutput=False)
    wb_d = nc.declare_dram_parameter("wb", [H, H], BF16, isOutput=False)
    wdrep_d = nc.declare_dram_parameter("wdrep", [1, 32 * H], BF16,
                                        isOutput=False)
    ind_d = nc.declare_dram_parameter("ind", [2, 32 * 512], BF16,
                                      isOutput=False)
    zws_d = nc.declare_dram_parameter("zws", [H, 63], BF16, isOutput=False)
    b1c_d = nc.declare_dram_parameter("b1c", [H, 1], F32, isOutput=False)
    corr3_d = nc.declare_dram_parameter("corr3", [128, 3], F32,
                                        isOutput=False)
    b256_d = nc.declare_dram_parameter("b256", [128, 1], F32, isOutput=False)
    v_d = nc.declare_dram_parameter("v", [N, 3], F32, isOutput=True)

    with tile.TileContext(nc) as tc:
        with (
            tc.tile_pool(name="const", bufs=1) as cpool,
            tc.tile_pool(name="work", bufs=2) as wpool,
            tc.tile_pool(name="silu", bufs=6) as lpool,
            tc.tile_pool(name="fin", bufs=2) as fpool,
            tc.tile_pool(name="pre", bufs=3, space="PSUM") as pre_pool,
            tc.tile_pool(name="cps", bufs=1, space="PSUM") as cps_pool,
            tc.tile_pool(name="bps", bufs=1, space="PSUM") as bps_pool,
        ):
            # ---- constants / inputs ----
            # SP queue: dist-critical path first (rep/pcol feed the dist
            # pipeline). hT/wb/wa go on the ACT queue, which is idle until
            # the sqrt, so both queues issue prologue DMAs in parallel.
            rep_all = cpool.tile([128, 3 * N], F32, tag="rep_all")
            nc.sync.dma_start(rep_all[:], rep_d[:])
            rep = [rep_all[:, N * a:N * (a + 1)] for a in range(3)]
            pcall = cpool.tile([128, 6], F32, tag="pcall")
            nc.sync.dma_start(pcall.rearrange("p (t c) -> p t c", t=2),
                              pos_d.rearrange("(t p) c -> p t c", t=2))
            pcol = [pcall[:, 3 * t:3 * t + 3] for t in range(2)]
            hT = cpool.tile([H, N], BF16, tag="hT")
            nc.scalar.dma_start(hT[:], hT_d[:])
            wb = cpool.tile([H, H], BF16, tag="wb")
            nc.scalar.dma_start(wb[:], wb_d[:])
            wa = cpool.tile([H, H], BF16, tag="wa")
            nc.scalar.dma_start(wa[:], wa_d[:])

            # Bd stationaries: per u, rows [wd; Bm[u]; Bm[u+128]] live at
            # partitions 32i..32i+2 (i=u//32), column block a=u%32.
            bwd4 = cpool.tile([128, 32 * H], BF16, tag="bwd4")
            # Bd moving rows: [dist_u ; ind_t0 ; ind_t1] same partitions,
            # block a at cols 512a..512a+512 = (t2, q).
            stage = cpool.tile([128, 32 * 512], BF16, tag="stage")
            for i in range(4):
                nc.sync.dma_start(bwd4[32 * i:32 * i + 1, :], wdrep_d[:])
                nc.sync.dma_start(stage[32 * i + 1:32 * i + 3, :], ind_d[:])

            # ---- dist: both halves in one [128, 512] set of tiles ----
            dx = wpool.tile([128, 512], F32, tag="dx")
            dy = wpool.tile([128, 512], F32, tag="dy")
            dz = wpool.tile([128, 512], F32, tag="dz")
            for t in range(2):
                nc.vector.tensor_scalar(dx[:, 256 * t:256 * t + 256],
                                        rep[0], pcol[t][:, 0:1],
                                        None, Alu.subtract)
                nc.vector.tensor_scalar(dy[:, 256 * t:256 * t + 256],
                                        rep[1], pcol[t][:, 1:2],
                                        None, Alu.subtract)
                nc.vector.tensor_scalar(dz[:, 256 * t:256 * t + 256],
                                        rep[2], pcol[t][:, 2:3],
                                        None, Alu.subtract)
            sx = wpool.tile([128, 512], F32, tag="sx")
            sy = wpool.tile([128, 512], F32, tag="sy")
            nc.vector.tensor_tensor(sx[:], dx[:], dx[:], Alu.mult)
            nc.vector.tensor_tensor(sy[:], dy[:], dy[:], Alu.mult)
            nc.vector.tensor_tensor(sx[:], sx[:], sy[:], Alu.add)
            nc.vector.tensor_tensor(sy[:], dz[:], dz[:], Alu.mult)
            nc.vector.tensor_tensor(sx[:], sx[:], sy[:], Alu.add)
            db = wpool.tile([128, 512], BF16, tag="db")
            nc.scalar.activation(db[:], sx[:], Act.Sqrt)

            # remaining small consts: ACT queue, after the sqrt
            zws = cpool.tile([H, 63], BF16, tag="zws")
            nc.scalar.dma_start(zws[:], zws_d[:])
            b1c = cpool.tile([H, 1], F32, tag="b1c")
            nc.scalar.dma_start(b1c[:], b1c_d[:])
            corr3 = cpool.tile([128, 3], F32, tag="corr3")
            nc.scalar.dma_start(corr3[:], corr3_d[:])
            b256 = cpool.tile([128, 1], F32, tag="b256")
            nc.scalar.dma_start(b256[:], b256_d[:])

            # ---- Bm = h @ Wb, scattered into bwd4 partitions 32i+1,32i+2 ----
            # copies on ScalarE (keeps DVE free for the dist chain)
            bsb = []
            for t in range(2):
                bp = bps_pool.tile([128, H], F32, tag="bps")
                nc.tensor.matmul(bp[:], hT[:, 128 * t:128 * (t + 1)],
                                 wb[:], start=True, stop=True,
                                 skip_group_check=True)
                bs = wpool.tile([128, H], BF16, tag="bsb", name=f"bsb{t}")
                nc.scalar.activation(bs[:], bp[:], Act.Copy)
                bsb.append(bs)
            # staging DMAs split across both issue queues (~600ns issue each)
            for t in range(2):
                for i in range(4):
                    dst = bwd4[32 * i + 1 + t:32 * i + 2 + t, :]
                    eng = nc.sync if (i % 2 == 0) else nc.scalar
                    eng.dma_start(
                        dst.rearrange("o (a c) -> o a c", c=H),
                        bsb[t][32 * i:32 * i + 32, :])
            for i in range(4):
                dst = stage[32 * i:32 * i + 1, :]
                dst = dst.rearrange("o (a two q) -> o a two q", two=2, q=N)
                eng = nc.sync if (i % 2 == 0) else nc.scalar
                eng.dma_start(dst[:],
                              db[32 * i:32 * i + 32, :].rearrange(
                                  "a (two q) -> a two q", two=2))

            # ---- coeff accumulator: partition u, cols (t2, q) ----
            cps = cps_pool.tile([128, 2 * N], F32, tag="c")

            rhs_a = hT[:, :].unsqueeze(1).broadcast_to([H, 2, N])

            def emit_w2(b):
                s0, s1 = sils[b]
                for i in range(4):
                    sil = s0 if i < 2 else s1
                    nc.tensor.matmul(cps[32 * i:32 * i + 32, :],
                                     zws[:, 31 - b:63 - b],
                                     sil[:, 512 * (i % 2):512 * (i % 2) + 512],
                                     start=(b == 0), stop=(b == 31),
                                     skip_group_check=True,
                                     tile_position=(0, 32 * i))

            sils = {}
            for a in range(32):
                pres = []
                for half in range(2):
                    pre = pre_pool.tile([128, 1024], F32, tag="pre",
                                        name=f"pre{half}_{a}")
                    for hh in range(2):
                        nc.tensor.matmul(pre[:, 512 * hh:512 * hh + 512],
                                         wa[:], rhs_a, start=True, stop=False,
                                         skip_group_check=True)
                    pres.append(pre)
                for i in range(4):
                    pre = pres[i // 2]
                    out_sl = pre[:, 512 * (i % 2):512 * (i % 2) + 512]
                    nc.tensor.matmul(out_sl,
                                     bwd4[32 * i:32 * i + 3,
                                          H * a:H * a + H],
                                     stage[32 * i:32 * i + 3,
                                           512 * a:512 * a + 512],
                                     start=False, stop=True,
                                     skip_group_check=True,
                                     tile_position=(32 * i, 0))
                cur = []
                for half in range(2):
                    sil = lpool.tile([128, 1024], BF16, tag="sil",
                                     name=f"sil{half}_{a}")
                    nc.scalar.activation(sil[:], pres[half][:], Act.Silu,
                                         bias=b1c[:, 0:1])
                    cur.append(sil)
                sils[a] = cur
                # software-pipeline: emit W2 for the PREVIOUS super-iteration
                if a > 0:
                    emit_w2(a - 1)
                if a == 31:
                    emit_w2(31)

            # ---- final: v = coeff @ pos - rowsum(coeff) * pos_p, with the
            # +b2 fold done via host-precomputed corr3 = b2*sum_q(pos) and
            # b256 = 256*b2: v = (cps@pos + corr3) - (rowsum(cps)+b256)*pos_p
            # ScalarE stages cps out of PSUM; t=0 reduces on DVE while t=1
            # reduces on GpSimd (which cannot read PSUM, hence the copy).
            for t in range(2):
                csb = fpool.tile([128, N], F32, tag="csb", name=f"csb{t}")
                nc.scalar.activation(csb[:], cps[:, N * t:N * (t + 1)],
                                     Act.Copy)
                vcol = fpool.tile([128, 3], F32, tag="vcol", name=f"vcol{t}")
                scr = fpool.tile([128, N], F32, tag="scr", name=f"scr{t}")
                for a in range(3):
                    nc.vector.scalar_tensor_tensor(
                        scr[:], csb[:], 1.0, rep[a], Alu.mult, Alu.mult,
                        accum_out=vcol[:, a:a + 1])
                rs = fpool.tile([128, 1], F32, tag="rs", name=f"rs{t}")
                nc.vector.tensor_scalar(scr[:], csb[:], 1.0, None,
                                        Alu.mult, Alu.add,
                                        accum_out=rs[:, 0:1])
                rs2 = fpool.tile([128, 1], F32, tag="rs2", name=f"rs2{t}")
                nc.vector.tensor_tensor(rs2[:], rs[:], b256[:], Alu.add)
                rsp = fpool.tile([128, 3], F32, tag="rsp", name=f"rsp{t}")
                nc.vector.tensor_scalar(rsp[:], pcol[t], rs2[:, 0:1],
                                        None, Alu.mult)
                vt = fpool.tile([128, 3], F32, tag="vt", name=f"vt{t}")
                nc.vector.tensor_tensor(vt[:], vcol[:], corr3[:], Alu.add)
                nc.vector.tensor_tensor(vt[:], vt[:], rsp[:], Alu.subtract)
                eng2 = nc.sync if t == 0 else nc.scalar
                eng2.dma_start(v_d[128 * t:128 * (t + 1), :], vt[:])

    nc.compile()
    return nc


def _prep_consts(W1, b1, W2, b2):
    import ml_dtypes
    bf16 = ml_dtypes.bfloat16

    wa = np.ascontiguousarray(W1[:H], dtype=bf16)
    wb = np.ascontiguousarray(W1[H:2 * H], dtype=bf16)
    wd = W1[2 * H].astype(np.float32)
    wdrep = np.ascontiguousarray(np.tile(wd, 32)[None, :]).astype(bf16)
    ind = np.zeros((2, 32 * 512), dtype=np.float32)
    cols = np.arange(32 * 512)
    ind[0, (cols % 512) < 256] = 1.0
    ind[1, (cols % 512) >= 256] = 1.0
    ind = ind.astype(bf16)
    zws = np.zeros((H, 63), dtype=np.float32)
    zws[:, 31] = W2[:, 0]
    zws = zws.astype(bf16)
    b1c = np.ascontiguousarray(b1.reshape(H, 1), dtype=np.float32)
    b2v = float(np.asarray(b2).reshape(-1)[0])
    b256 = np.full((128, 1), 256.0 * b2v, dtype=np.float32)
    return dict(wa=wa, wb=wb, wdrep=wdrep, ind=ind, zws=zws, b1c=b1c,
                b256=b256, b2v=b2v)


def _make_in_maps(h, pos, consts):
    import ml_dtypes
    bf16 = ml_dtypes.bfloat16

    consts = dict(consts)
    b2v = consts.pop("b2v")
    in_maps = []
    for g in range(B):
        hg = h[g * N:(g + 1) * N]
        pg = pos[g * N:(g + 1) * N]
        rep3 = np.ascontiguousarray(
            np.broadcast_to(pg.T.reshape(1, 3 * N), (128, 3 * N)),
            dtype=np.float32)
        corr3 = np.ascontiguousarray(
            np.broadcast_to((b2v * pg.sum(axis=0))[None, :], (128, 3)),
            dtype=np.float32)
        m = {"hT": np.ascontiguousarray(hg.T).astype(bf16), "pos": pg,
             "rep3": rep3, "corr3": corr3}
        m.update(consts)
        in_maps.append(m)
    return in_maps


def kernel(h, pos, batch, W1, b1, W2, b2, **unused):
    from concourse.bass_utils import run_bass_kernel_spmd

    h = np.ascontiguousarray(np.asarray(h, dtype=np.float32))
    pos = np.ascontiguousarray(np.asarray(pos, dtype=np.float32))
    W1 = np.asarray(W1, dtype=np.float32)
    b1 = np.asarray(b1, dtype=np.float32)
    W2 = np.asarray(W2, dtype=np.float32)
    b2 = np.asarray(b2, dtype=np.float32)

    if "nc" not in _cache:
        _cache["nc"] = _build()
    nc = _cache["nc"]

    consts = _prep_consts(W1, b1, W2, b2)
    in_maps = _make_in_maps(h, pos, consts)
    res = run_bass_kernel_spmd(nc, in_maps, core_ids=list(range(NCORES)))
    return np.concatenate([r["v"] for r in res.results], axis=0)


# revision 17
# speedup vs baseline: 1.9763x; 1.0210x over previous
"""Trainium2 Bass kernel for nn_EquivariantVelocityHead.

Full-input contract: kernel(**inputs) takes the unsharded inputs (as in
setup_inputs()) and returns the full [B*N, 3] output. Internally shards
data-parallel over the graph dimension B across 8 NeuronCores (all pairwise
interactions are intra-graph), with the tiny phi-MLP weights replicated.

Math (per graph, N=256 nodes, H=128):
  A = h @ W1[:H];  Bm = h @ W1[H:2H];  wd = W1[2H];  (phi layer 1 split)
  pre[p,q,:] = A[q] + Bm[p] + dist[p,q]*wd + b1
  coeff[p,q] = silu(pre) @ W2 + b2
  v[p] = sum_q coeff[p,q] * (pos[q] - pos[p])
       = coeff @ pos - rowsum(coeff) * pos[p]

v2 device layout (all matmul operands bf16 so FWL fast-weight-load kicks in):
  Node pairing: u in [0,128) covers nodes (u, u+128); a 512-col block is
  (t2, q) with t2 the pair half. Super-iteration a in [0,32) processes the
  four u's {a, a+32, a+64, a+96} (index i = u//32):
   - A-pass: lhsT=Wa, rhs = hT broadcast [H,2,N], one 512-col MM per u,
     start of the PSUM accumulation for that bank.
   - Bd-pass: K=3 stationary [wd; Bm[u]; Bm[u+128]] at partitions 32i..32i+2
     with moving rows [dist_u; ind_t0; ind_t1]; the four u's go to the four
     PE row groups (tile_position=(32i,0)) so they run concurrently.
   - silu+b1 fused on ScalarE reading PSUM, bf16 out.
   - W2-pass: stationary [128,32] slice of a sliding (zeros|W2|zeros) tile,
     at column group i (tile_position=(0,32i)); node-pair u accumulates into
     cps partition u (= a within group i). Four groups run concurrently.
  Final contraction v = coeff@pos - rowsum(coeff)*pos on VectorE in fp32.
"""
import sys

sys.path.insert(0, "/opt/trn_rl_repo")

import numpy as np

B, N, H = 8, 256, 128
NCORES = 8

_cache = {}


def _build():
    import concourse.bacc as bacc
    import concourse.mybir as mybir
    import concourse.tile as tile

    F32 = mybir.dt.float32
    BF16 = mybir.dt.bfloat16
    Alu = mybir.AluOpType
    Act = mybir.ActivationFunctionType

    nc = bacc.Bacc()

    hT_d = nc.declare_dram_parameter("hT", [H, N], BF16, isOutput=False)
    pos_d = nc.declare_dram_parameter("pos", [N, 3], F32, isOutput=False)
    rep_d = nc.declare_dram_parameter("rep3", [128, 3 * N], BF16,
                                      isOutput=False)
    repf_d = nc.declare_dram_parameter("repf", [128, 3 * N], F32,
                                       isOutput=False)
    wa_d = nc.declare_dram_parameter("wa", [H, H], BF16, isOutput=False)
    wb_d = nc.declare_dram_parameter("wb", [H, H], BF16, isOutput=False)
    wdrep_d = nc.declare_dram_parameter("wdrep", [1, 32 * H], BF16,
                                        isOutput=False)
    ind_d = nc.declare_dram_parameter("ind", [2, 32 * 512], BF16,
                                      isOutput=False)
    zws_d = nc.declare_dram_parameter("zws", [H, 63], BF16, isOutput=False)
    b1c_d = nc.declare_dram_parameter("b1c", [H, 1], F32, isOutput=False)
    corr3_d = nc.declare_dram_parameter("corr3", [128, 3], F32,
                                        isOutput=False)
    b256_d = nc.declare_dram_parameter("b256", [128, 1], F32, isOutput=False)
    v_d = nc.declare_dram_parameter("v", [N, 3], F32, isOutput=True)

    with tile.TileContext(nc) as tc:
        with (
            tc.tile_pool(name="const", bufs=1) as cpool,
            tc.tile_pool(name="work", bufs=2) as wpool,
            tc.tile_pool(name="silu", bufs=6) as lpool,
            tc.tile_pool(name="fin", bufs=2) as fpool,
            tc.tile_pool(name="pre", bufs=3, space="PSUM") as pre_pool,
            tc.tile_pool(name="cps", bufs=1, space="PSUM") as cps_pool,
            tc.tile_pool(name="bps", bufs=1, space="PSUM") as bps_pool,
        ):
            # ---- constants / inputs ----
            # SP queue: dist-critical path first (rep/pcol feed the dist
            # pipeline). hT/wb/wa + ind go on the ACT queue, which is idle
            # until the sqrt, so both queues issue prologue DMAs in parallel.
            rep_all = cpool.tile([128, 3 * N], BF16, tag="rep_all")
            nc.sync.dma_start(rep_all[:], rep_d[:])
            rep = [rep_all[:, N * a:N * (a + 1)] for a in range(3)]
            pcall = cpool.tile([128, 6], F32, tag="pcall")
            nc.sync.dma_start(pcall.rearrange("p (t c) -> p t c", t=2),
                              pos_d.rearrange("(t p) c -> p t c", t=2))
            pcol = [pcall[:, 3 * t:3 * t + 3] for t in range(2)]
            hT = cpool.tile([H, N], BF16, tag="hT")
            nc.scalar.dma_start(hT[:], hT_d[:])
            wb = cpool.tile([H, H], BF16, tag="wb")
            nc.scalar.dma_start(wb[:], wb_d[:])
            wa = cpool.tile([H, H], BF16, tag="wa")
            nc.scalar.dma_start(wa[:], wa_d[:])

            # Bd stationaries: per u, rows [wd; Bm[u]; Bm[u+128]] live at
            # partitions 32i..32i+2 (i=u//32), column block a=u%32.
            bwd4 = cpool.tile([128, 32 * H], BF16, tag="bwd4")
            # Bd moving rows: [dist_u ; ind_t0 ; ind_t1] same partitions,
            # block a at cols 512a..512a+512 = (t2, q).
            stage = cpool.tile([128, 32 * 512], BF16, tag="stage")
            for i in range(4):
                nc.sync.dma_start(bwd4[32 * i:32 * i + 1, :], wdrep_d[:])
                nc.scalar.dma_start(stage[32 * i + 1:32 * i + 3, :],
                                    ind_d[:])

            # fp32 pos broadcast for the epilogue reduce: loaded lazily, off
            # the prologue critical path.
            repf_all = cpool.tile([128, 3 * N], F32, tag="repf_all")
            nc.sync.dma_start(repf_all[:], repf_d[:])
            repf = [repf_all[:, N * a:N * (a + 1)] for a in range(3)]

            # ---- dist: both halves in one [128, 512] set of tiles (bf16,
            # so the DVE runs in 2x perf mode) ----
            dx = wpool.tile([128, 512], BF16, tag="dx")
            dy = wpool.tile([128, 512], BF16, tag="dy")
            dz = wpool.tile([128, 512], BF16, tag="dz")
            for t in range(2):
                nc.vector.tensor_scalar(dx[:, 256 * t:256 * t + 256],
                                        rep[0], pcol[t][:, 0:1],
                                        None, Alu.subtract)
                nc.vector.tensor_scalar(dy[:, 256 * t:256 * t + 256],
                                        rep[1], pcol[t][:, 1:2],
                                        None, Alu.subtract)
                nc.vector.tensor_scalar(dz[:, 256 * t:256 * t + 256],
                                        rep[2], pcol[t][:, 2:3],
                                        None, Alu.subtract)
            sx = wpool.tile([128, 512], BF16, tag="sx")
            sy = wpool.tile([128, 512], BF16, tag="sy")
            nc.vector.tensor_tensor(sx[:], dx[:], dx[:], Alu.mult)
            nc.vector.tensor_tensor(sy[:], dy[:], dy[:], Alu.mult)
            nc.vector.tensor_tensor(sx[:], sx[:], sy[:], Alu.add)
            nc.vector.tensor_tensor(sy[:], dz[:], dz[:], Alu.mult)
            nc.vector.tensor_tensor(sx[:], sx[:], sy[:], Alu.add)
            db = wpool.tile([128, 512], BF16, tag="db")
            nc.scalar.activation(db[:], sx[:], Act.Sqrt)

            # remaining small consts: ACT queue, after the sqrt
            zws = cpool.tile([H, 63], BF16, tag="zws")
            nc.scalar.dma_start(zws[:], zws_d[:])
            b1c = cpool.tile([H, 1], F32, tag="b1c")
            nc.scalar.dma_start(b1c[:], b1c_d[:])
            corr3 = cpool.tile([128, 3], F32, tag="corr3")
            nc.scalar.dma_start(corr3[:], corr3_d[:])
            b256 = cpool.tile([128, 1], F32, tag="b256")
            nc.scalar.dma_start(b256[:], b256_d[:])

            # ---- Bm = h @ Wb, scattered into bwd4 partitions 32i+1,32i+2 ----
            # copies on ScalarE (keeps DVE free for the dist chain)
            bsb = []
            for t in range(2):
                bp = bps_pool.tile([128, H], F32, tag="bps")
                nc.tensor.matmul(bp[:], hT[:, 128 * t:128 * (t + 1)],
                                 wb[:], start=True, stop=True,
                                 skip_group_check=True)
                bs = wpool.tile([128, H], BF16, tag="bsb", name=f"bsb{t}")
                nc.scalar.activation(bs[:], bp[:], Act.Copy)
                bsb.append(bs)
            # staging DMAs split across both issue queues (~600ns issue each)
            for t in range(2):
                for i in range(4):
                    dst = bwd4[32 * i + 1 + t:32 * i + 2 + t, :]
                    eng = nc.sync if (i % 2 == 0) else nc.scalar
                    eng.dma_start(
                        dst.rearrange("o (a c) -> o a c", c=H),
                        bsb[t][32 * i:32 * i + 32, :])
            for i in range(4):
                dst = stage[32 * i:32 * i + 1, :]
                dst = dst.rearrange("o (a two q) -> o a two q", two=2, q=N)
                eng = nc.sync if (i % 2 == 0) else nc.scalar
                eng.dma_start(dst[:],
                              db[32 * i:32 * i + 32, :].rearrange(
                                  "a (two q) -> a two q", two=2))

            # ---- coeff accumulator: partition u, cols (t2, q) ----
            cps = cps_pool.tile([128, 2 * N], F32, tag="c")

            rhs_a = hT[:, :].unsqueeze(1).broadcast_to([H, 2, N])

            def emit_w2(b):
                s0, s1 = sils[b]
                for i in range(4):
                    sil = s0 if i < 2 else s1
                    nc.tensor.matmul(cps[32 * i:32 * i + 32, :],
                                     zws[:, 31 - b:63 - b],
                                     sil[:, 512 * (i % 2):512 * (i % 2) + 512],
                                     start=(b == 0), stop=(b == 31),
                                     skip_group_check=True,
                                     tile_position=(0, 32 * i))

            sils = {}
            for a in range(32):
                pres = []
                for half in range(2):
                    pre = pre_pool.tile([128, 1024], F32, tag="pre",
                                        name=f"pre{half}_{a}")
                    for hh in range(2):
                        nc.tensor.matmul(pre[:, 512 * hh:512 * hh + 512],
                                         wa[:], rhs_a, start=True, stop=False,
                                         skip_group_check=True)
                    pres.append(pre)
                for i in range(4):
                    pre = pres[i // 2]
                    out_sl = pre[:, 512 * (i % 2):512 * (i % 2) + 512]
                    nc.tensor.matmul(out_sl,
                                     bwd4[32 * i:32 * i + 3,
                                          H * a:H * a + H],
                                     stage[32 * i:32 * i + 3,
                                           512 * a:512 * a + 512],
                                     start=False, stop=True,
                                     skip_group_check=True,
                                     tile_position=(32 * i, 0))
                cur = []
                for half in range(2):
                    sil = lpool.tile([128, 1024], BF16, tag="sil",
                                     name=f"sil{half}_{a}")
                    nc.scalar.activation(sil[:], pres[half][:], Act.Silu,
                                         bias=b1c[:, 0:1])
                    cur.append(sil)
                sils[a] = cur
                # software-pipeline: emit W2 for the PREVIOUS super-iteration
                if a > 0:
                    emit_w2(a - 1)
                if a == 31:
                    emit_w2(31)

            # ---- final: v = coeff @ pos - rowsum(coeff) * pos_p, with the
            # +b2 fold done via host-precomputed corr3 = b2*sum_q(pos) and
            # b256 = 256*b2: v = (cps@pos + corr3) - (rowsum(cps)+b256)*pos_p
            # ScalarE stages cps out of PSUM; t=0 reduces on DVE while t=1
            # reduces on GpSimd (which cannot read PSUM, hence the copy).
            for t in range(2):
                csb = fpool.tile([128, N], F32, tag="csb", name=f"csb{t}")
                nc.scalar.activation(csb[:], cps[:, N * t:N * (t + 1)],
                                     Act.Copy)
                vcol = fpool.tile([128, 3], F32, tag="vcol", name=f"vcol{t}")
                scr = fpool.tile([128, N], F32, tag="scr", name=f"scr{t}")
                for a in range(3):
                    nc.vector.scalar_tensor_tensor(
                        scr[:], csb[:], 1.0, repf[a], Alu.mult, Alu.mult,
                        accum_out=vcol[:, a:a + 1])
                rs = fpool.tile([128, 1], F32, tag="rs", name=f"rs{t}")
                nc.vector.tensor_scalar(scr[:], csb[:], 1.0, None,
                                        Alu.mult, Alu.add,
                                        accum_out=rs[:, 0:1])
                rs2 = fpool.tile([128, 1], F32, tag="rs2", name=f"rs2{t}")
                nc.vector.tensor_tensor(rs2[:], rs[:], b256[:], Alu.add)
                rsp = fpool.tile([128, 3], F32, tag="rsp", name=f"rsp{t}")
                nc.vector.tensor_scalar(rsp[:], pcol[t], rs2[:, 0:1],
                                        None, Alu.mult)
                vt = fpool.tile([128, 3], F32, tag="vt", name=f"vt{t}")
                nc.vector.tensor_tensor(vt[:], vcol[:], corr3[:], Alu.add)
                nc.vector.tensor_tensor(vt[:], vt[:], rsp[:], Alu.subtract)
                eng2 = nc.sync if t == 0 else nc.scalar
                eng2.dma_start(v_d[128 * t:128 * (t + 1), :], vt[:])

    nc.compile()
    return nc


def _prep_consts(W1, b1, W2, b2):
    import ml_dtypes
    bf16 = ml_dtypes.bfloat16

    wa = np.ascontiguousarray(W1[:H], dtype=bf16)
    wb = np.ascontiguousarray(W1[H:2 * H], dtype=bf16)
    wd = W1[2 * H].astype(np.float32)
    wdrep = np.ascontiguousarray(np.tile(wd, 32)[None, :]).astype(bf16)
    ind = np.zeros((2, 32 * 512), dtype=np.float32)
    cols = np.arange(32 * 512)
    ind[0, (cols % 512) < 256] = 1.0
    ind[1, (cols % 512) >= 256] = 1.0
    ind = ind.astype(bf16)
    zws = np.zeros((H, 63), dtype=np.float32)
    zws[:, 31] = W2[:, 0]
    zws = zws.astype(bf16)
    b1c = np.ascontiguousarray(b1.reshape(H, 1), dtype=np.float32)
    b2v = float(np.asarray(b2).reshape(-1)[0])
    b256 = np.full((128, 1), 256.0 * b2v, dtype=np.float32)
    return dict(wa=wa, wb=wb, wdrep=wdrep, ind=ind, zws=zws, b1c=b1c,
                b256=b256, b2v=b2v)


def _make_in_maps(h, pos, consts):
    import ml_dtypes
    bf16 = ml_dtypes.bfloat16

    consts = dict(consts)
    b2v = consts.pop("b2v")
    in_maps = []
    for g in range(B):
        hg = h[g * N:(g + 1) * N]
        pg = pos[g * N:(g + 1) * N]
        repf = np.ascontiguousarray(
            np.broadcast_to(pg.T.reshape(1, 3 * N), (128, 3 * N)),
            dtype=np.float32)
        corr3 = np.ascontiguousarray(
            np.broadcast_to((b2v * pg.sum(axis=0))[None, :], (128, 3)),
            dtype=np.float32)
        m = {"hT": np.ascontiguousarray(hg.T).astype(bf16), "pos": pg,
             "rep3": repf.astype(bf16), "repf": repf, "corr3": corr3}
        m.update(consts)
        in_maps.append(m)
    return in_maps


def kernel(h, pos, batch, W1, b1, W2, b2, **unused):
    from concourse.bass_utils import run_bass_kernel_spmd

    h = np.ascontiguousarray(np.asarray(h, dtype=np.float32))
    pos = np.ascontiguousarray(np.asarray(pos, dtype=np.float32))
    W1 = np.asarray(W1, dtype=np.float32)
    b1 = np.asarray(b1, dtype=np.float32)
    W2 = np.asarray(W2, dtype=np.float32)
    b2 = np.asarray(b2, dtype=np.float32)

    if "nc" not in _cache:
        _cache["nc"] = _build()
    nc = _cache["nc"]

    consts = _prep_consts(W1, b1, W2, b2)
    in_maps = _make_in_maps(h, pos, consts)
    res = run_bass_kernel_spmd(nc, in_maps, core_ids=list(range(NCORES)))
    return np.concatenate([r["v"] for r in res.results], axis=0)
